# revision 1
# baseline (speedup 1.0000x reference)
"""A3TGCN (2-layer TGCN + temporal attention) distributed Bass kernel for
8 Trainium2 NeuronCores.

Math restructuring (validated vs reference to ~6e-7 in fp64):
  - PyG GCNConv:  gcn(h, Wc, bc) = Ahat @ (h Wc) + bc  where
      Ahat = D^-1/2 (A_w + I) D^-1/2  (self loops appended as edges).
    Associativity: Ahat (h Wc) = (Ahat h) Wc, so the three gates of a TGCN
    cell share ONE sparse aggregation  s = Ahat h.
  - Gate algebra folds to   z = sigmoid(s A_z + h B_z + c_z)  etc, with
      A_g = Wc_g Wl_g[:H],  B_g = Wl_g[H:],  c_g = bc_g Wl_g[:H] + bl_g.
  - Layer-0 aggregation inputs are x_t (no recurrence) and layer-1 inputs
    are layer-0 outputs, so the 32 aggregations batch into TWO passes over
    the edges with T*H = 2048 features each.

Distribution: nodes sharded 1250/core (graph parallel). Aggregation output
(dst) rows are core-local; gather sources are the full node table (x is
replicated via in_maps; h0 is AllGather'ed across the 8 cores). Edge gather
uses indirect DMA of 4KB bf16 rows, deduplicated per 2-dst-tile super-group;
the weighted scatter-add is a matmul with host-built 128x(2*128) scatter
blocks accumulated in PSUM.

GRU/attention run in (H=128 partitions, nodes free) layout; PE transposes
(identity matmul) convert aggregation output (dst, feat) -> (feat, dst) and
hidden states back to node-major rows for the pass-B gather source.
"""
import numpy as np
import ml_dtypes

import concourse.bass as bass
import concourse.tile as tile
from concourse import bacc, mybir
from concourse.bass_utils import run_bass_kernel_spmd
from concourse.masks import make_identity

# problem constants
N, E, F, H, T, L, O = 10000, 320000, 128, 128, 16, 2, 128
P = 128
N_CORES = 8
N_LOCAL = N // N_CORES            # 1250
N_TILES = (N_LOCAL + P - 1) // P  # 10
N_PAD = N_TILES * P               # 1280
S_TILES = 2                       # dst-tiles per dedup super-group
N_SUPER = N_TILES // S_TILES      # 5
C = T * H                         # 2048 features per aggregation pass
NCH = 4                           # 512-col matmul chunks per dst tile
CHUNKS = [(0, 512), (512, 512), (1024, 256)]  # (start, len) over N_PAD
H1N = 512             # node-half 1 size (local ids [0, H1N) -> AllGather-1)
H2N = N_PAD - H1N     # node-half 2 size (768)
CHUNKS_H1 = [(0, 512)]
CHUNKS_H2 = [(512, 512), (1024, 256)]
SGB = 8     # groups per sg DMA batch
IDB = 16    # groups per idx DMA batch
BF16 = mybir.dt.bfloat16
F32 = mybir.dt.float32


# ----------------------------------------------------------------- host prep
def _prep_graph(edge_index, edge_weight):
    src = np.asarray(edge_index[0], np.int64)
    dst = np.asarray(edge_index[1], np.int64)
    ew = np.asarray(edge_weight, np.float64)

    deg = np.zeros(N)
    np.add.at(deg, dst, ew)
    deg += 1.0
    dinv = 1.0 / np.sqrt(deg)
    norm_e = (dinv[src] * ew * dinv[dst]).astype(np.float32)
    self_norm = (dinv * dinv).astype(np.float32)
    src_all = np.concatenate([src, np.arange(N)])
    dst_all = np.concatenate([dst, np.arange(N)])
    w_all = np.concatenate([norm_e, self_norm]).astype(np.float32)
    core_of = dst_all // N_LOCAL

    # Unique srcs per (core, super-group), split into "low" (local id < H1N,
    # delivered by AllGather-1) and "high" (delivered by AllGather-2) so
    # pass-B low-groups can gather while AG-2 is still in flight.
    uniq = {}
    for c in range(N_CORES):
        m = core_of == c
        s, d, w = src_all[m], dst_all[m] - c * N_LOCAL, w_all[m]
        sup = d // (S_TILES * P)
        for u in range(N_SUPER):
            mu = sup == u
            su, du, wu = s[mu], d[mu] - u * S_TILES * P, w[mu]
            us, inv = np.unique(su, return_inverse=True)
            lowmask = (us % N_LOCAL) < H1N
            # reorder uniques: low first, then high
            order = np.argsort(~lowmask, kind="stable")
            rank = np.empty_like(order)
            rank[order] = np.arange(len(us))
            uniq[(c, u)] = (us[order], rank[inv], du, wu, int(lowmask.sum()))

    groups_low = np.zeros(N_SUPER, np.int64)
    groups_high = np.zeros(N_SUPER, np.int64)
    for u in range(N_SUPER):
        groups_low[u] = max((uniq[(c, u)][4] + P - 1) // P for c in range(N_CORES))
        groups_high[u] = max(
            (len(uniq[(c, u)][0]) - uniq[(c, u)][4] + P - 1) // P for c in range(N_CORES))
    G = int(groups_low.sum() + groups_high.sum())

    idx_a = np.zeros((N_CORES, G, P), np.int32)   # rows of x table (N rows)
    idx_b = np.zeros((N_CORES, G, P), np.int32)   # rows of h0 low/high tables
    sg_all = np.zeros((N_CORES, G, P, S_TILES * P), np.float32)
    g_starts = np.concatenate([[0], np.cumsum(groups_low + groups_high)])
    for c in range(N_CORES):
        for u in range(N_SUPER):
            us, inv, du, wu, nlow = uniq[(c, u)]
            g0 = int(g_starts[u])
            glow = int(groups_low[u])
            # low uniques at group slots [0, nlow); high uniques start at
            # slot glow*P (so low/high group blocks don't mix)
            slot = np.where(np.arange(len(us)) < nlow,
                            np.arange(len(us)),
                            glow * P + np.arange(len(us)) - nlow)
            fa = idx_a[c].reshape(-1)
            fa[g0 * P + slot] = us
            rb, loc = us // N_LOCAL, us % N_LOCAL
            fb = idx_b[c].reshape(-1)
            fb[g0 * P + slot] = np.where(loc < H1N, rb * H1N + loc,
                                         rb * H2N + (loc - H1N))
            eslot = slot[inv]
            grp, row = eslot // P, eslot % P
            np.add.at(sg_all[c], (g0 + grp, row, du), wu)
    groups_per_super = groups_low + groups_high

    # batched-contiguous layouts for efficient DMA:
    #   idx: (NBI, P, IDB) — batch b col k = group b*IDB+k
    #   sg:  (NBS, P, SGB*S_TILES*P) — batch b block k = group b*SGB+k
    nbi = (G + IDB - 1) // IDB
    nbs = (G + SGB - 1) // SGB
    def batch_idx(idx):
        pad = np.zeros((N_CORES, nbi * IDB, P), np.int32)
        pad[:, :G] = idx
        return np.ascontiguousarray(pad.reshape(N_CORES, nbi, IDB, P).transpose(0, 1, 3, 2))
    sg_pad = np.zeros((N_CORES, nbs * SGB, P, S_TILES * P), sg_all.dtype)
    sg_pad[:, :G] = sg_all
    sg_b = np.ascontiguousarray(
        sg_pad.reshape(N_CORES, nbs, SGB, P, S_TILES * P).transpose(0, 1, 3, 2, 4)
        .reshape(N_CORES, nbs, P, SGB * S_TILES * P))
    return (groups_low, groups_high, batch_idx(idx_a), batch_idx(idx_b),
            sg_b.astype(ml_dtypes.bfloat16))


def _fold_weights(inp):
    """Wpack (128, 12*128) bf16, lhsT blocks ordered [l][gate z,r,h][A|B];
    biases (128, 6) f32, col = l*3 + gate."""
    W = np.zeros((12, H, H), np.float64)
    Bias = np.zeros((H, 6), np.float64)
    for l in range(L):
        for gi, g in enumerate("zrh"):
            Wc = np.asarray(inp[f"Wc{g}"][l], np.float64)
            bc = np.asarray(inp[f"bc{g}"][l], np.float64)
            Wl = np.asarray(inp[f"Wl{g}"][l], np.float64)
            bl = np.asarray(inp[f"bl{g}"][l], np.float64)
            W[l * 6 + gi * 2] = Wc @ Wl[:H]       # A_g
            W[l * 6 + gi * 2 + 1] = Wl[H:]        # B_g
            Bias[:, l * 3 + gi] = bc @ Wl[:H] + bl
    Wpack = np.transpose(W, (1, 0, 2)).reshape(H, 12 * H)
    return Wpack.astype(ml_dtypes.bfloat16), Bias.astype(np.float32)


# -------------------------------------------------------------- device build
def _build_program(groups_low, groups_high):
    groups_per_super = groups_low + groups_high
    G = int(groups_per_super.sum())
    nc = bacc.Bacc("TRN2", target_bir_lowering=False, debug=False, num_devices=N_CORES)

    NBI = (G + IDB - 1) // IDB
    NBS = (G + SGB - 1) // SGB
    x_dram = nc.dram_tensor("xsrc", [N, C], BF16, kind="ExternalInput")
    idxa_dram = nc.dram_tensor("idxa", [NBI, P, IDB], mybir.dt.int32, kind="ExternalInput")
    idxb_dram = nc.dram_tensor("idxb", [NBI, P, IDB], mybir.dt.int32, kind="ExternalInput")
    sg_dram = nc.dram_tensor("sg", [NBS, P, SGB * S_TILES * P], BF16, kind="ExternalInput")
    w_dram = nc.dram_tensor("wpack", [H, 12 * H], BF16, kind="ExternalInput")
    b_dram = nc.dram_tensor("bias", [H, 6], F32, kind="ExternalInput")
    attw_dram = nc.dram_tensor("attw", [H, 1], BF16, kind="ExternalInput")
    outw_dram = nc.dram_tensor("outw", [H, O], BF16, kind="ExternalInput")
    outb_dram = nc.dram_tensor("outb", [O, 1], F32, kind="ExternalInput")
    out_dram = nc.dram_tensor("out", [N_PAD, O], F32, kind="ExternalOutput")

    h0_localL = nc.dram_tensor("h0_localL", [H1N, C], BF16)
    h0_localH = nc.dram_tensor("h0_localH", [H2N, C], BF16)
    h0_fullL = nc.dram_tensor("h0_fullL", [N_CORES * H1N, C], BF16, addr_space="Shared")
    h0_fullH = nc.dram_tensor("h0_fullH", [N_CORES * H2N, C], BF16, addr_space="Shared")

    with tile.TileContext(nc) as tc:
        with (
            tc.tile_pool(name="const", bufs=1) as constp,
            tc.tile_pool(name="big", bufs=1) as bigp,
            tc.tile_pool(name="gat", bufs=4) as gatp,
            tc.tile_pool(name="idxp", bufs=6) as idxp,
            tc.tile_pool(name="sgp", bufs=4) as sgp,
            tc.tile_pool(name="work", bufs=2) as workp,
            tc.tile_pool(name="state", bufs=2) as statep,
        ):
            # ---- constants / weights
            id_bf = constp.tile([P, P], BF16, name="id_bf")
            make_identity(nc, id_bf[:])
            id_f32 = constp.tile([P, P], F32, name="id_f32")
            make_identity(nc, id_f32[:])
            ones_bf = constp.tile([1, P], BF16, name="ones_bf")
            nc.gpsimd.memset(ones_bf[:], 1.0)
            wsb = constp.tile([H, 12 * H], BF16, name="wsb")
            nc.sync.dma_start(out=wsb[:], in_=w_dram[:])
            bsb = constp.tile([H, 6], F32, name="bsb")
            nc.sync.dma_start(out=bsb[:], in_=b_dram[:])
            attw_sb = constp.tile([H, 1], BF16, name="attw_sb")
            nc.sync.dma_start(out=attw_sb[:], in_=attw_dram[:])
            outw_sb = constp.tile([H, O], BF16, name="outw_sb")
            nc.sync.dma_start(out=outw_sb[:], in_=outw_dram[:])
            outb_sb = constp.tile([O, 1], F32, name="outb_sb")
            nc.sync.dma_start(out=outb_sb[:], in_=outb_dram[:])
            hzero = constp.tile([H, N_PAD], BF16, name="hzero")
            nc.gpsimd.memset(hzero[:], 0.0)

            # ---- persistent big buffers (tag-shared across phases)
            s0_sb = bigp.tile([P, N_TILES * C], BF16, name="s0_sb", tag="sfeat")
            s0T_sb = bigp.tile([H, T * N_PAD], BF16, name="s0T_sb", tag="sT")
            h0T_sb = bigp.tile([P, N_TILES * C], BF16, name="h0T_sb", tag="hrows")

            def w_ap(l, gate, which):  # lhsT block
                k = l * 6 + gate * 2 + which
                return wsb[:, k * H:(k + 1) * H]

            def bias_ap(l, gate):
                return bsb[:, l * 3 + gate:l * 3 + gate + 1]

            # =================== aggregation pass =====================
            # sg/idx loads are batched (SGB/IDB groups per DMA) so the HWDGE
            # queue doesn't flood the shared SDMA engines with 512B descriptors
            def agg_round(tag, idx_dram_, src_of, dst_big, accp, rng_of, accumulate):
                """One accumulation round over all supers. rng_of(u) gives the
                global group range; accumulate=True adds the psum result onto
                dst_big (used by the high round on top of the low partials)."""
                for u in range(N_SUPER):
                    glo, ghi = rng_of(u)
                    if glo >= ghi:
                        continue
                    acc = [accp.tile([P, 512], F32, tag=f"acc{s}_{i}",
                                     name=f"acc{tag}_{u}_{s}_{i}")
                           for s in range(S_TILES) for i in range(NCH)]
                    for gi in range(glo, ghi):
                        uid = f"{tag}_{u}_{gi}"
                        if gi % IDB == 0 or gi == glo:
                            idx_t = idxp.tile([P, IDB], mybir.dt.int32,
                                              name=f"idx_{tag}_{gi}", tag="idx")
                            nc.sync.dma_start(out=idx_t[:], in_=idx_dram_[gi // IDB])
                        sg_t = sgp.tile([P, S_TILES * P], BF16,
                                        name=f"sg_{uid}", tag="sg")
                        nc.sync.dma_start(
                            out=sg_t[:],
                            in_=sg_dram[gi // SGB, :, (gi % SGB) * S_TILES * P:
                                        (gi % SGB + 1) * S_TILES * P])
                        gat_t = gatp.tile([P, C], BF16, name=f"gat_{uid}", tag="gat")
                        nc.gpsimd.indirect_dma_start(
                            out=gat_t[:], out_offset=None, in_=src_of(u)[:],
                            in_offset=bass.IndirectOffsetOnAxis(
                                ap=idx_t[:, gi % IDB: gi % IDB + 1], axis=0))
                        for s in range(S_TILES):
                            for ch in range(NCH):
                                nc.tensor.matmul(
                                    acc[s * NCH + ch][:],
                                    lhsT=sg_t[:, s * P:(s + 1) * P],
                                    rhs=gat_t[:, ch * 512:(ch + 1) * 512],
                                    start=(gi == glo), stop=(gi == ghi - 1))
                    for s in range(S_TILES):
                        t_ = u * S_TILES + s
                        for ch in range(NCH):
                            dst = dst_big[:, t_ * C + ch * 512: t_ * C + (ch + 1) * 512]
                            if accumulate:
                                nc.vector.tensor_tensor(out=dst, in0=dst,
                                                        in1=acc[s * NCH + ch][:],
                                                        op=mybir.AluOpType.add)
                            else:
                                nc.vector.tensor_copy(dst, acc[s * NCH + ch][:])

            g_starts = np.concatenate([[0], np.cumsum(groups_per_super)])

            def agg_pass(tag, idx_dram_, src_low, src_high, dst_big, two_round):
                with tc.tile_pool(name=f"acc{tag}", bufs=1, space="PSUM") as accp:
                    if two_round:
                        agg_round(tag + "L", idx_dram_, lambda u: src_low, dst_big, accp,
                                  lambda u: (int(g_starts[u]), int(g_starts[u] + groups_low[u])),
                                  accumulate=False)
                        agg_round(tag + "H", idx_dram_, lambda u: src_high, dst_big, accp,
                                  lambda u: (int(g_starts[u] + groups_low[u]), int(g_starts[u + 1])),
                                  accumulate=True)
                    else:
                        def src_of(u):
                            return src_low
                        agg_round(tag, idx_dram_, src_of, dst_big, accp,
                                  lambda u: (int(g_starts[u]), int(g_starts[u + 1])),
                                  accumulate=False)

            # =================== transpose s -> sT ====================
            def transpose_s(tag, src_big, dstT_big, tpp):
                # t-major so GRU step t can start after its 10 tiles
                for t_ in range(T):
                    for tile_ in range(N_TILES):
                        tp = tpp.tile([P, P], BF16, tag="tp", bufs=2,
                                      name=f"tp{tag}_{t_}_{tile_}")
                        nc.tensor.transpose(
                            tp[:], src_big[:, tile_ * C + t_ * H: tile_ * C + (t_ + 1) * H],
                            id_bf[:])
                        nc.vector.tensor_copy(
                            dstT_big[:, t_ * N_PAD + tile_ * P: t_ * N_PAD + (tile_ + 1) * P],
                            tp[:])

            # =================== GRU layer ============================
            def gru_layer(l, sT_big, gpp, emit_h, chunks=CHUNKS):
                """emit_h(t) -> (Tile, col0) destination for the (H, N_PAD)
                hidden state of step t. Yields (t, (Tile, col0))."""
                hprev, hcol = hzero, 0
                for t_ in range(T):
                    hnew, ncol = emit_h(t_)
                    for ci, (c0, cl) in enumerate(chunks):
                        sT = sT_big[:, t_ * N_PAD + c0: t_ * N_PAD + c0 + cl]
                        hp_ap = hprev[:, hcol + c0: hcol + c0 + cl]
                        zp = gpp.tile([P, cl], F32, tag="gz", bufs=1, name=f"gz{l}_{t_}_{ci}")
                        nc.tensor.matmul(zp[:], lhsT=w_ap(l, 0, 0), rhs=sT, start=True, stop=False)
                        nc.tensor.matmul(zp[:], lhsT=w_ap(l, 0, 1), rhs=hp_ap, start=False, stop=True)
                        z_sb = workp.tile([P, cl], BF16, tag="z_sb", bufs=2, name=f"z{l}_{t_}_{ci}")
                        nc.scalar.activation(z_sb[:], zp[:], mybir.ActivationFunctionType.Sigmoid,
                                             bias=bias_ap(l, 0))
                        rp = gpp.tile([P, cl], F32, tag="gr", bufs=1, name=f"gr{l}_{t_}_{ci}")
                        nc.tensor.matmul(rp[:], lhsT=w_ap(l, 1, 0), rhs=sT, start=True, stop=False)
                        nc.tensor.matmul(rp[:], lhsT=w_ap(l, 1, 1), rhs=hp_ap, start=False, stop=True)
                        r_sb = workp.tile([P, cl], BF16, tag="r_sb", bufs=2, name=f"r{l}_{t_}_{ci}")
                        nc.scalar.activation(r_sb[:], rp[:], mybir.ActivationFunctionType.Sigmoid,
                                             bias=bias_ap(l, 1))
                        rh = workp.tile([P, cl], BF16, tag="rh", bufs=2, name=f"rh{l}_{t_}_{ci}")
                        nc.vector.tensor_tensor(out=rh[:], in0=r_sb[:], in1=hp_ap,
                                                op=mybir.AluOpType.mult)
                        hp_ = gpp.tile([P, cl], F32, tag="gh", bufs=1, name=f"gh{l}_{t_}_{ci}")
                        nc.tensor.matmul(hp_[:], lhsT=w_ap(l, 2, 0), rhs=sT, start=True, stop=False)
                        nc.tensor.matmul(hp_[:], lhsT=w_ap(l, 2, 1), rhs=rh[:], start=False, stop=True)
                        ht = workp.tile([P, cl], BF16, tag="ht", bufs=2, name=f"ht{l}_{t_}_{ci}")
                        nc.scalar.activation(ht[:], hp_[:], mybir.ActivationFunctionType.Tanh,
                                             bias=bias_ap(l, 2))
                        # h_new = ht + z*(hprev - ht)
                        df = workp.tile([P, cl], BF16, tag="df", bufs=2, name=f"df{l}_{t_}_{ci}")
                        nc.vector.tensor_tensor(out=df[:], in0=hp_ap, in1=ht[:],
                                                op=mybir.AluOpType.subtract)
                        zd = workp.tile([P, cl], BF16, tag="zd", bufs=2, name=f"zd{l}_{t_}_{ci}")
                        nc.vector.tensor_tensor(out=zd[:], in0=z_sb[:], in1=df[:],
                                                op=mybir.AluOpType.mult)
                        nc.vector.tensor_tensor(out=hnew[:, ncol + c0: ncol + c0 + cl],
                                                in0=ht[:], in1=zd[:], op=mybir.AluOpType.add)
                    hprev, hcol = hnew, ncol
                    yield t_, hnew, ncol

            # ============== PASS A ==============
            agg_pass("a", idxa_dram, x_dram, x_dram, s0_sb, two_round=False)

            with tc.tile_pool(name="tailA", bufs=1, space="PSUM") as tpp:
                transpose_s("a", s0_sb, s0T_sb, tpp)

                # GRU layer 0 per node-half so AllGather-1 (node-local ids
                # [0,H1N) of every core) can fly while half-2 still computes.
                def run_l0_half(half, chunks, tiles, h0_loc, h0_ful):
                    def emit_h0(t_):
                        h = statep.tile([H, N_PAD], BF16, tag=f"h0state{half}",
                                        bufs=2, name=f"h0_{half}_{t_}")
                        return h, 0
                    for t_, hnew, ncol in gru_layer(0, s0T_sb, tpp, emit_h0, chunks):
                        for tile_ in tiles:
                            tp = tpp.tile([P, P], BF16, tag="tph", bufs=2,
                                          name=f"tph_{half}_{t_}_{tile_}")
                            nc.tensor.transpose(
                                tp[:], hnew[:, ncol + tile_ * P: ncol + (tile_ + 1) * P],
                                id_bf[:])
                            nc.vector.tensor_copy(
                                h0T_sb[:, tile_ * C + t_ * H: tile_ * C + (t_ + 1) * H], tp[:])
                    for k, tile_ in enumerate(tiles):
                        nc.scalar.dma_start(out=h0_loc[k * P:(k + 1) * P, :],
                                            in_=h0T_sb[:, tile_ * C:(tile_ + 1) * C])
                    nc.gpsimd.collective_compute(
                        "AllGather", mybir.AluOpType.bypass,
                        replica_groups=[list(range(N_CORES))],
                        ins=[h0_loc[:, :]], outs=[h0_ful[:, :]])

                run_l0_half(0, CHUNKS_H1, range(0, H1N // P), h0_localL, h0_fullL)
                run_l0_half(1, CHUNKS_H2, range(H1N // P, N_TILES), h0_localH, h0_fullH)

            # ============== PASS B ==============
            s1_sb = bigp.tile([P, N_TILES * C], BF16, name="s1_sb", tag="sfeat")
            s1T_sb = bigp.tile([H, T * N_PAD], BF16, name="s1T_sb", tag="sT")
            agg_pass("b", idxb_dram, h0_fullL, h0_fullH, s1_sb, two_round=True)

            h1all = bigp.tile([H, T * N_PAD], BF16, name="h1all", tag="hrows")
            # Attention: softmax needs no max-subtraction: |h1|<=1 (convex
            # combos of tanh) and att_W in [0,1) bound |score| <= sum(att_W)
            # ~ 64+-3.3, well inside f32/bf16 exp range; att_b only shifts
            # scores (softmax is shift-invariant) so it is dropped.
            # ctx_u accumulates sum_t h1_t * exp(s_t), den accumulates
            # sum_t exp(s_t); normalized at the end.
            ctx_sb = workp.tile([H, N_PAD], BF16, tag="ctx", bufs=1, name="ctx_sb")
            den = workp.tile([1, N_PAD], BF16, tag="den", bufs=1, name="den")

            with tc.tile_pool(name="tailB", bufs=1, space="PSUM") as tpp:
                transpose_s("b", s1_sb, s1T_sb, tpp)

                def emit_h1(t_):
                    return h1all, t_ * N_PAD

                for t_, hnew, ncol in gru_layer(1, s1T_sb, tpp, emit_h1):
                    e_t = workp.tile([1, N_PAD], BF16, tag="e_t", bufs=2, name=f"e_{t_}")
                    for ci, (c0, cl) in enumerate(CHUNKS):
                        sp = tpp.tile([1, cl], F32, tag="misc", bufs=2, name=f"sc_{t_}_{ci}")
                        nc.tensor.matmul(sp[:], lhsT=attw_sb[:],
                                         rhs=hnew[:, ncol + c0: ncol + c0 + cl],
                                         start=True, stop=True)
                        nc.scalar.activation(e_t[0:1, c0:c0 + cl], sp[:],
                                             mybir.ActivationFunctionType.Exp)
                    bc = workp.tile([H, N_PAD], BF16, tag="bc", bufs=2, name=f"bc_{t_}")
                    nc.gpsimd.partition_broadcast(bc[:], e_t[0:1, :])
                    tmp = workp.tile([H, N_PAD], BF16, tag="ctmp", bufs=2, name=f"ctmp_{t_}")
                    nc.vector.tensor_tensor(
                        out=tmp[:], in0=h1all[:, t_ * N_PAD:(t_ + 1) * N_PAD],
                        in1=bc[:], op=mybir.AluOpType.mult)
                    if t_ == 0:
                        nc.vector.tensor_copy(ctx_sb[:], tmp[:])
                        nc.vector.tensor_copy(den[:], e_t[:])
                    else:
                        nc.vector.tensor_tensor(out=ctx_sb[:], in0=ctx_sb[:],
                                                in1=tmp[:], op=mybir.AluOpType.add)
                        nc.vector.tensor_tensor(out=den[:], in0=den[:], in1=e_t[:],
                                                op=mybir.AluOpType.add)

                # normalize: ctx *= broadcast(1/den)
                rinv_bf = workp.tile([1, N_PAD], BF16, tag="rinvb", bufs=1, name="rinv_bf")
                with nc.allow_low_precision(reason="softmax denom reciprocal, bf16 is enough"):
                    nc.vector.reciprocal(rinv_bf[:], den[:])
                rb = workp.tile([H, N_PAD], BF16, tag="bc", bufs=2, name="rb")
                nc.gpsimd.partition_broadcast(rb[:], rinv_bf[0:1, :])
                nc.vector.tensor_tensor(out=ctx_sb[:], in0=ctx_sb[:], in1=rb[:],
                                        op=mybir.AluOpType.mult)

                # out = (ctx^T out_W + out_b)^T
                oT_sb = workp.tile([O, N_PAD], F32, tag="oT", bufs=1, name="oT_sb")
                for ci, (c0, cl) in enumerate(CHUNKS):
                    op_ = tpp.tile([P, cl], F32, tag="gz", bufs=1, name=f"op_{ci}")
                    nc.tensor.matmul(op_[:], lhsT=outw_sb[:], rhs=ctx_sb[:, c0:c0 + cl],
                                     start=True, stop=True)
                    nc.scalar.activation(oT_sb[:, c0:c0 + cl], op_[:],
                                         mybir.ActivationFunctionType.Identity,
                                         bias=outb_sb[:, :1])
                for tile_ in range(N_TILES):
                    tp = tpp.tile([P, P], F32, tag="tp", bufs=2, name=f"ot_{tile_}")
                    nc.tensor.transpose(tp[:], oT_sb[:, tile_ * P:(tile_ + 1) * P], id_f32[:])
                    ot = workp.tile([P, P], F32, tag="otsb", bufs=2, name=f"otsb_{tile_}")
                    nc.vector.tensor_copy(ot[:], tp[:])
                    nc.scalar.dma_start(out=out_dram[tile_ * P:(tile_ + 1) * P, :], in_=ot[:])

    nc.compile()
    return nc


_CACHE = {}


def _get_program(groups_low, groups_high):
    key = (tuple(int(v) for v in groups_low), tuple(int(v) for v in groups_high))
    if key not in _CACHE:
        _CACHE[key] = _build_program(groups_low, groups_high)
    return _CACHE[key]


def make_in_maps(inputs):
    x = np.asarray(inputs["x"], np.float32)
    edge_index = np.asarray(inputs["edge_index"])
    edge_weight = np.asarray(inputs["edge_weight"], np.float32)

    groups_low, groups_high, idx_a, idx_b, sg_all = _prep_graph(edge_index, edge_weight)
    wpack, bias = _fold_weights(inputs)
    xb = np.ascontiguousarray(np.transpose(x, (0, 2, 1)).reshape(N, C)).astype(ml_dtypes.bfloat16)
    attw = np.asarray(inputs["att_W"], np.float32).reshape(H, 1).astype(ml_dtypes.bfloat16)
    outw = np.asarray(inputs["out_W"], np.float32).astype(ml_dtypes.bfloat16)
    outb = np.asarray(inputs["out_b"], np.float32).reshape(O, 1)

    in_maps = [
        {
            "xsrc": xb,
            "idxa": idx_a[c],
            "idxb": idx_b[c],
            "sg": sg_all[c],
            "wpack": wpack,
            "bias": bias,
            "attw": attw,
            "outw": outw,
            "outb": outb,
        }
        for c in range(N_CORES)
    ]
    return (groups_low, groups_high), in_maps


def kernel(**inputs) -> np.ndarray:
    (groups_low, groups_high), in_maps = make_in_maps(inputs)
    nc = _get_program(groups_low, groups_high)
    res = run_bass_kernel_spmd(nc, in_maps, core_ids=list(range(N_CORES)))
    out = np.concatenate([res.results[c]["out"][:N_LOCAL] for c in range(N_CORES)], axis=0)
    return out.astype(np.float32)

